# revision 3
# baseline (speedup 1.0000x reference)
"""MoE (single shared expert) kernel for 8 trn2 NeuronCores — fp8 DoubleRow.

Math: the reference's top-2 gating over 64 "experts" feeds a single shared
FFN, and the renormalized top-2 weights sum to s/(s+1e-9) with s >= 1/64,
i.e. 1 up to <= 6.4e-8 relative -- below f32 rounding noise.  The whole
module therefore reduces to:  out = silu(x @ up_w.T) @ down_w.T.

Precision: fp8e4 (ml_dtypes.float8_e4m3, max 240) matmuls in DoubleRow
perf mode (2 k-tiles per instruction, 0.5 PE cycles/row).  Naive fp8
quantization gives ~5.4e-2 rel err (tolerance 2e-2), so every GEMM runs
THREE DoubleRow passes accumulating into the same PSUM bank:
    A: hi @ hi      B: hi @ w_lo      C: x_lo @ hi
where t_lo = q8(t - q8(t)) is an UNBOOSTED fp8 residual (same scale, so
no per-pass rescaling is needed).  Measured rel err of 3+3 passes on the
real inputs: 1.9e-3 (beats all-bf16's 3.3e-3).

Scales: up_w is stored as q8(32*up_w) (std 0.02 would be subnormal in
fp8), down_w as q8(64*down_w).  h = psum1*sigmoid(psum1/32) then carries
the 32x scale into GEMM2 automatically; final output is descaled by
1/(32*64) in the scalar-engine Copy that evacuates PSUM2.

Sharding (8 cores): token-parallel, 1024 tokens per core; weights
replicated.  Host pre-quantizes and pre-interleaves everything into the
[pair*128, 2, free] DoubleRow layout, so device DMAs are plain
contiguous loads.
"""

import os
import sys

import numpy as np
import ml_dtypes

for _p in ("/opt/trn_rl_repo",):
    if os.path.isdir(_p) and _p not in sys.path:
        sys.path.insert(0, _p)

import concourse.bass as bass
import concourse.mybir as mybir
import concourse.tile as tile

F32 = mybir.dt.float32
F8 = mybir.dt.float8e4
E4M3 = ml_dtypes.float8_e4m3
DR = mybir.MatmulPerfMode.DoubleRow


def _ensure_axon_hooks_shim():
    """bass_utils' trace path imports antenv.axon_hooks, which this image
    lacks; give it a no-op hook module so BASS_TRACE=1 degrades gracefully."""
    import types
    if "antenv.axon_hooks" in sys.modules:
        return
    try:
        import antenv
    except ImportError:
        return
    if hasattr(antenv, "axon_hooks"):
        return
    ah = types.ModuleType("antenv.axon_hooks")
    ah._hook = None
    ah.set_axon_ntff_profile_hook = lambda h: setattr(ah, "_hook", h)
    ah.get_axon_ntff_profile_hook = lambda: ah._hook
    sys.modules["antenv.axon_hooks"] = ah
    antenv.axon_hooks = ah


_ensure_axon_hooks_shim()


def _split_multi_waits(nc):
    """This container's walrus encodes at most ONE sync wait per engine
    instruction ("Too many sync wait commands").  Tile routinely emits
    instructions waiting on several semaphores; hoist the extra waits onto
    single-wait NoOps inserted just before, on the same engine."""
    n = 0
    for f in nc.m.functions:
        for blk in f.blocks:
            insts = blk.instructions
            out = []
            for inst in insts:
                si = inst.sync_info
                waits = list(si.on_wait) if si and si.on_wait else []
                if len(waits) > 1:
                    for w in waits[:-1]:
                        n += 1
                        nop = mybir.InstNoOp(name=f"I-wsplit-{n}", ins=[], outs=[])
                        nop.engine = inst.engine
                        nop.sync_info = mybir.SyncInfo(on_wait=[w], on_update=[])
                        nc.register_instruction(nop)
                        out.append(nop)
                    si.on_wait = [waits[-1]]
                out.append(inst)
            if n:
                insts[:] = out
    return n


# Problem shape (hardcoded per contract)
B, S, D, ED = 4, 2048, 1024, 2048
T = B * S                     # 8192 tokens
NC_CORES = 8
TC = T // NC_CORES            # tokens per core = 1024
TT = 512                      # token tile (matmul moving free dim)
NTT = TC // TT                # 2 token tiles
KP1 = D // 256                # 4 DoubleRow k-pairs for GEMM1
KP2 = ED // 256               # 8 DoubleRow k-pairs for GEMM2
M1 = ED // 128                # 16 output e-tiles of GEMM1
M2 = D // 128                 # 8 output d-tiles of GEMM2
SU = 32.0                     # up_w pre-scale
SD = 64.0                     # down_w pre-scale

_CACHE = {}
LAST_RESULTS = None           # BassKernelResults of the most recent run


def build_nc(np1: int = 3, np2: int = 3) -> bass.Bass:
    """One-core SPMD program: ytp[D, TC] = (silu(x @ upT) @ dwnT).T
    with np1/np2 fp8 DoubleRow passes for GEMM1/GEMM2."""
    nc = bass.Bass()
    xhi = nc.dram_tensor("xhi", [KP1 * 128, 2, TC], F8, kind="ExternalInput")
    xlo = nc.dram_tensor("xlo", [KP1 * 128, 2, TC], F8, kind="ExternalInput")
    uphi = nc.dram_tensor("uphi", [KP1 * 128, 2, ED], F8, kind="ExternalInput")
    uplo = nc.dram_tensor("uplo", [KP1 * 128, 2, ED], F8, kind="ExternalInput")
    dnhi = nc.dram_tensor("dnhi", [KP2 * 128, 2, D], F8, kind="ExternalInput")
    dnlo = nc.dram_tensor("dnlo", [KP2 * 128, 2, D], F8, kind="ExternalInput")
    ytp = nc.dram_tensor("ytp", [D, TC], F32, kind="ExternalOutput")

    with tile.TileContext(nc) as tc:
        with (
            tc.tile_pool(name="wpool", bufs=1) as wpool,
            tc.tile_pool(name="hpool", bufs=8) as hpool,
            tc.tile_pool(name="ypool", bufs=4) as ypool,
            tc.tile_pool(name="psum", bufs=8, space="PSUM") as psum,
        ):
            up_hi_sb = [wpool.tile([128, 2, ED], F8, tag=f"uh{p}", name=f"uh{p}") for p in range(KP1)]
            up_lo_sb = [wpool.tile([128, 2, ED], F8, tag=f"ul{p}", name=f"ul{p}") for p in range(KP1)]
            x_hi_sb = [wpool.tile([128, 2, TC], F8, tag=f"xh{p}", name=f"xh{p}") for p in range(KP1)]
            x_lo_sb = [wpool.tile([128, 2, TC], F8, tag=f"xl{p}", name=f"xl{p}") for p in range(KP1)]
            dn_hi_sb = [wpool.tile([128, 2, D], F8, tag=f"dh{p}", name=f"dh{p}") for p in range(KP2)]
            dn_lo_sb = [wpool.tile([128, 2, D], F8, tag=f"dl{p}", name=f"dl{p}") for p in range(KP2)]
            # h tiles: [128, 2, TT] per (token tile, k-pair of GEMM2)
            h_hi_sb = [[wpool.tile([128, 2, TT], F8, tag=f"hh{tt}_{p}", name=f"hh{tt}_{p}")
                        for p in range(KP2)] for tt in range(NTT)]
            h_lo_sb = [[wpool.tile([128, 2, TT], F8, tag=f"hl{tt}_{p}", name=f"hl{tt}_{p}")
                        for p in range(KP2)] for tt in range(NTT)]

            def dma(sb, dram, p):
                nc.sync.dma_start(out=sb[:], in_=dram[p * 128:(p + 1) * 128, :, :])

            # DMA emission in consumption order: per-pair groups of
            # (up_hi, up_lo, x_hi, x_lo) = 1.5MB so the PE's first
            # m-tile group (pair-major A,B,C chain) unblocks after ~4us.
            for p in range(KP1):
                dma(up_hi_sb[p], uphi, p)
                if np1 >= 2:
                    dma(up_lo_sb[p], uplo, p)
                dma(x_hi_sb[p], xhi, p)
                if np1 >= 3:
                    dma(x_lo_sb[p], xlo, p)
            for p in range(KP2):
                dma(dn_hi_sb[p], dnhi, p)
                if np2 >= 2:
                    dma(dn_lo_sb[p], dnlo, p)

            g1_passes = [(x_hi_sb, up_hi_sb), (x_hi_sb, up_lo_sb),
                         (x_lo_sb, up_hi_sb)][:np1]

            def gemm1(tt, ei):
                t0 = tt * TT
                ps = psum.tile([128, TT], F32, tag="ps", name=f"ps1_{tt}_{ei}")
                n_mm = KP1 * len(g1_passes)
                k = 0
                for p in range(KP1):
                    for (xs, us) in g1_passes:
                        nc.tensor.matmul(
                            ps[:],
                            us[p][:, :, ei * 128:(ei + 1) * 128],
                            xs[p][:, :, t0:t0 + TT],
                            start=(k == 0),
                            stop=(k == n_mm - 1),
                            perf_mode=DR,
                        )
                        k += 1
                sg = hpool.tile([128, TT], F32, tag="sg", bufs=3, name=f"sg_{tt}_{ei}")
                nc.scalar.activation(
                    sg[:], ps[:], mybir.ActivationFunctionType.Sigmoid,
                    scale=1.0 / SU,
                )
                hs = hpool.tile([128, TT], F32, tag="hs", bufs=3, name=f"hs_{tt}_{ei}")
                nc.vector.tensor_mul(hs[:], ps[:], sg[:])
                hh = h_hi_sb[tt][ei // 2][:, ei % 2, :]
                nc.vector.tensor_copy(hh, hs[:])
                if np2 >= 3:
                    hl = h_lo_sb[tt][ei // 2][:, ei % 2, :]
                    # hl = (hs * 1.0) - hh   (one DVE op)
                    nc.vector.scalar_tensor_tensor(
                        hl, hs[:], 1.0, hh,
                        op0=mybir.AluOpType.mult,
                        op1=mybir.AluOpType.subtract,
                    )

            g2_passes = [("hh", dn_hi_sb), ("hh", dn_lo_sb),
                         ("hl", dn_hi_sb)][:np2]

            def gemm2(tt, db):
                t0 = tt * TT
                ps = psum.tile([128, TT], F32, tag="ps", name=f"ps2_{tt}_{db}")
                n_mm = KP2 * len(g2_passes)
                k = 0
                for p in range(KP2):
                    for (hk, ds) in g2_passes:
                        hsb = h_hi_sb[tt][p] if hk == "hh" else h_lo_sb[tt][p]
                        nc.tensor.matmul(
                            ps[:],
                            ds[p][:, :, db * 128:(db + 1) * 128],
                            hsb[:, :, :],
                            start=(k == 0),
                            stop=(k == n_mm - 1),
                            perf_mode=DR,
                        )
                        k += 1
                y = ypool.tile([128, TT], F32, tag="y", name=f"y_{tt}_{db}")
                nc.scalar.activation(
                    y[:], ps[:], mybir.ActivationFunctionType.Copy,
                    scale=1.0 / (SU * SD),
                )
                nc.sync.dma_start(
                    out=ytp[db * 128:(db + 1) * 128, t0:t0 + TT], in_=y[:],
                )

            for ei in range(M1):
                gemm1(0, ei)
            for ei in range(M1):
                gemm1(1, ei)
            for db in range(M2):
                gemm2(0, db)
            for db in range(M2):
                gemm2(1, db)

    _split_multi_waits(nc)
    nc.finalize()
    return nc


def _get_nc(np1: int, np2: int) -> bass.Bass:
    key = (np1, np2)
    if key not in _CACHE:
        _CACHE[key] = build_nc(np1, np2)
    return _CACHE[key]


def _q8(a: np.ndarray) -> np.ndarray:
    return np.clip(a, -240.0, 240.0).astype(E4M3)


def _dr_layout(a: np.ndarray, npair: int) -> np.ndarray:
    """[K, F] -> [npair*128, 2, F] with [p*128+k, i, f] = a[256p+128i+k, f]."""
    K, F = a.shape
    assert K == npair * 256
    return np.ascontiguousarray(
        a.reshape(npair, 2, 128, F).transpose(0, 2, 1, 3).reshape(npair * 128, 2, F)
    )


def kernel(x, gate_w, up_w, down_w):
    global LAST_RESULTS
    from concourse.bass_utils import run_bass_kernel_spmd

    np1, np2 = (int(c) for c in os.environ.get("MOE_PASSES", "3+3").split("+"))
    nc = _get_nc(np1, np2)

    xf = np.asarray(x, dtype=np.float32).reshape(T, D)
    up = np.asarray(up_w, dtype=np.float32)
    dn = np.asarray(down_w, dtype=np.float32)

    # Quantize once on the full tensors (hi + unboosted residual lo).
    xT = np.ascontiguousarray(xf.T)                    # [D, T]
    x_hi = _q8(xT)
    x_lo = _q8(xT - x_hi.astype(np.float32))
    upT = np.ascontiguousarray((SU * up).T)            # [D, ED]
    up_hi = _q8(upT)
    up_lo = _q8(upT - up_hi.astype(np.float32))
    dnT = np.ascontiguousarray((SD * dn).T)            # [ED, D]
    dn_hi = _q8(dnT)
    dn_lo = _q8(dnT - dn_hi.astype(np.float32))

    up_hi_d = _dr_layout(up_hi, KP1)
    up_lo_d = _dr_layout(up_lo, KP1)
    dn_hi_d = _dr_layout(dn_hi, KP2)
    dn_lo_d = _dr_layout(dn_lo, KP2)

    in_maps = []
    for c in range(NC_CORES):
        cols = slice(c * TC, (c + 1) * TC)
        in_maps.append({
            "xhi": _dr_layout(x_hi[:, cols], KP1),
            "xlo": _dr_layout(x_lo[:, cols], KP1),
            "uphi": up_hi_d,
            "uplo": up_lo_d,
            "dnhi": dn_hi_d,
            "dnlo": dn_lo_d,
        })

    res = run_bass_kernel_spmd(nc, in_maps, list(range(NC_CORES)))
    LAST_RESULTS = res

    out = np.empty((T, D), dtype=np.float32)
    for c in range(NC_CORES):
        out[c * TC:(c + 1) * TC, :] = res.results[c]["ytp"].T
    return out.reshape(B, S, D)


# revision 4
# speedup vs baseline: 1.2141x; 1.2141x over previous
"""MoE (single shared expert) kernel for 8 trn2 NeuronCores — bf16 token-parallel.

Math: the reference's top-2 gating over 64 "experts" feeds a single shared
FFN, and the renormalized top-2 weights sum to s/(s+1e-9) with s >= 1/64,
i.e. 1 up to <= 6.4e-8 relative -- below f32 rounding noise.  The whole
module therefore reduces to:  out = silu(x @ up_w.T) @ down_w.T.

Why bf16: TRN2's PE retires one 512-col matmul instruction per ~518
cycles for f32r, bf16 AND fp8-DoubleRow alike (measured 216ns issue
spacing), so the 512-instruction PE floor is ~110.6us per core no matter
the dtype.  (fp8 DoubleRow doubles MACs/instr but needs 6 total
residual-corrected passes to meet 2e-2 rel err -- measured 5.4e-2 naive
-- so it loses.)  bf16 keeps the floor while halving DMA bytes, which is
what the remaining 28us of baseline overhead was made of:
  - 14.3us start delay (f32 weight DMA prefix)  -> ~3us (0.75MB prefix)
  - 11.6us tail (8MB f32 partial-output drain)  -> ~1us (2MB bf16 out)

Sharding (8 cores): token-parallel, 1024 tokens/core, weights replicated
(up 4MB + dn 4MB bf16), x shard 2MB, out 2MB bf16 upcast on host.
GEMM1 for the first 8 m-tiles runs as k-quarter sweeps so the PE starts
after only (up[0..1], x[0..1]) = 1.5MB of DMA; everything after is
straight-line m-major with k inner.
"""

import os
import sys

import numpy as np
import ml_dtypes

for _p in ("/opt/trn_rl_repo",):
    if os.path.isdir(_p) and _p not in sys.path:
        sys.path.insert(0, _p)

import concourse.bass as bass
import concourse.mybir as mybir
import concourse.tile as tile

F32 = mybir.dt.float32
BF16 = mybir.dt.bfloat16
NP_BF16 = ml_dtypes.bfloat16


def _ensure_axon_hooks_shim():
    """bass_utils' trace path imports antenv.axon_hooks, which this image
    lacks; give it a no-op hook module so BASS_TRACE=1 degrades gracefully."""
    import types
    if "antenv.axon_hooks" in sys.modules:
        return
    try:
        import antenv
    except ImportError:
        return
    if hasattr(antenv, "axon_hooks"):
        return
    ah = types.ModuleType("antenv.axon_hooks")
    ah._hook = None
    ah.set_axon_ntff_profile_hook = lambda h: setattr(ah, "_hook", h)
    ah.get_axon_ntff_profile_hook = lambda: ah._hook
    sys.modules["antenv.axon_hooks"] = ah
    antenv.axon_hooks = ah


_ensure_axon_hooks_shim()


def _split_multi_waits(nc):
    """This container's walrus encodes at most ONE sync wait per engine
    instruction ("Too many sync wait commands").  Tile routinely emits
    instructions waiting on several semaphores; hoist the extra waits onto
    single-wait NoOps inserted just before, on the same engine."""
    n = 0
    for f in nc.m.functions:
        for blk in f.blocks:
            insts = blk.instructions
            out = []
            for inst in insts:
                si = inst.sync_info
                waits = list(si.on_wait) if si and si.on_wait else []
                if len(waits) > 1:
                    for w in waits[:-1]:
                        n += 1
                        nop = mybir.InstNoOp(name=f"I-wsplit-{n}", ins=[], outs=[])
                        nop.engine = inst.engine
                        nop.sync_info = mybir.SyncInfo(on_wait=[w], on_update=[])
                        nc.register_instruction(nop)
                        out.append(nop)
                    si.on_wait = [waits[-1]]
                out.append(inst)
            if n:
                insts[:] = out
    return n


# Problem shape (hardcoded per contract)
B, S, D, ED = 4, 2048, 1024, 2048
T = B * S                     # 8192 tokens
NC_CORES = 8
TC = T // NC_CORES            # tokens per core = 1024
TT = 512                      # token tile (matmul moving free dim)
NTT = TC // TT                # 2 token tiles
NK1 = D // 128                # 8 k-tiles for GEMM1
NK2 = ED // 128               # 16 k-tiles for GEMM2
M1 = ED // 128                # 16 output e-tiles of GEMM1
M2 = D // 128                 # 8 output d-tiles of GEMM2

_CACHE = {}
LAST_RESULTS = None           # BassKernelResults of the most recent run


def build_nc(warm: int = 0) -> bass.Bass:
    """One-core SPMD program: ytp[D, TC] = (silu(x @ upT) @ dwnT).T (bf16)."""
    nc = bass.Bass()
    xt = nc.dram_tensor("xt", [D, TC], BF16, kind="ExternalInput")
    upw = nc.dram_tensor("upw", [D, ED], BF16, kind="ExternalInput")
    dwn = nc.dram_tensor("dwn", [ED, D], BF16, kind="ExternalInput")
    ytp = nc.dram_tensor("ytp", [D, TC], BF16, kind="ExternalOutput")

    with tile.TileContext(nc) as tc:
        with (
            tc.tile_pool(name="wpool", bufs=1) as wpool,
            tc.tile_pool(name="hpool", bufs=6) as hpool,
            tc.tile_pool(name="ypool", bufs=4) as ypool,
            tc.tile_pool(name="psum", bufs=8, space="PSUM") as psum,
        ):
            up_sb = [wpool.tile([128, ED], BF16, tag=f"up{k}", name=f"up{k}")
                     for k in range(NK1)]
            x_sb = [wpool.tile([128, TC], BF16, tag=f"x{k}", name=f"x{k}")
                    for k in range(NK1)]
            dn_sb = [wpool.tile([128, D], BF16, tag=f"dn{k}", name=f"dn{k}")
                     for k in range(NK2)]
            h_sb = [[wpool.tile([128, TT], BF16, tag=f"h{tt}_{e}", name=f"h{tt}_{e}")
                     for e in range(M1)] for tt in range(NTT)]

            # Optional PE pre-ramp: a dense block of dependency-free matmuls
            # issued at t=0 so the HAM clock reaches 2.4GHz before real work.
            if warm:
                wz = wpool.tile([128, 128], BF16, tag="warmw", name="warmw")
                xz = wpool.tile([128, TT], BF16, tag="warmx", name="warmx")
                nc.vector.memset(wz[:], 0.0)
                nc.vector.memset(xz[:], 0.0)
                wps = psum.tile([128, TT], F32, tag="ps", name="warm_ps")
                for _ in range(warm):
                    nc.tensor.matmul(wps[:], wz[:], xz[:], start=True, stop=True)
                wsink = ypool.tile([128, TT], F32, tag="wsink", name="warm_sink")
                nc.vector.tensor_copy(wsink[:], wps[:])

            # DMA emission in consumption order.
            for k in range(NK1):
                nc.sync.dma_start(out=up_sb[k][:], in_=upw[k * 128:(k + 1) * 128, :])
                nc.sync.dma_start(out=x_sb[k][:], in_=xt[k * 128:(k + 1) * 128, :])
            for k in range(NK2):
                nc.sync.dma_start(out=dn_sb[k][:], in_=dwn[k * 128:(k + 1) * 128, :])

            def evac1(tt, ei, ps):
                sg = hpool.tile([128, TT], F32, tag="sg", bufs=4,
                                name=f"sg_{tt}_{ei}")
                nc.scalar.activation(
                    sg[:], ps[:], mybir.ActivationFunctionType.Sigmoid,
                )
                nc.vector.tensor_mul(h_sb[tt][ei][:], ps[:], sg[:])

            def gemm1_plain(tt, eis):
                t0 = tt * TT
                for ei in eis:
                    ps = psum.tile([128, TT], F32, tag="ps", name=f"ps1_{tt}_{ei}")
                    for k in range(NK1):
                        nc.tensor.matmul(
                            ps[:],
                            up_sb[k][:, ei * 128:(ei + 1) * 128],
                            x_sb[k][:, t0:t0 + TT],
                            start=(k == 0),
                            stop=(k == NK1 - 1),
                        )
                    evac1(tt, ei, ps)

            def gemm1_ksweep(tt, eis, kchunk=2):
                """k-chunked sweeps across len(eis) concurrent PSUM banks so
                the first matmul only needs up[0:kchunk] + x[0:kchunk]."""
                t0 = tt * TT
                pss = [psum.tile([128, TT], F32, tag="ps", name=f"ps1_{tt}_{ei}")
                       for ei in eis]
                for k0 in range(0, NK1, kchunk):
                    for j, ei in enumerate(eis):
                        for k in range(k0, k0 + kchunk):
                            nc.tensor.matmul(
                                pss[j][:],
                                up_sb[k][:, ei * 128:(ei + 1) * 128],
                                x_sb[k][:, t0:t0 + TT],
                                start=(k == 0),
                                stop=(k == NK1 - 1),
                            )
                for j, ei in enumerate(eis):
                    evac1(tt, ei, pss[j])

            def gemm2(tt, dbs, split_last=False):
                t0 = tt * TT
                for db in dbs:
                    if split_last and db == dbs[-1]:
                        # shorten the kernel tail: column halves so the first
                        # half's evac+DMA overlap the second half's matmuls
                        dsl = slice(db * 128, (db + 1) * 128)
                        half = TT // 2
                        for hx in range(2):
                            ps = psum.tile([128, half], F32, tag="ps",
                                           name=f"ps2_last_{hx}")
                            for k in range(NK2):
                                nc.tensor.matmul(
                                    ps[:],
                                    dn_sb[k][:, dsl],
                                    h_sb[tt][k][:, hx * half:(hx + 1) * half],
                                    start=(k == 0),
                                    stop=(k == NK2 - 1),
                                )
                            y = ypool.tile([128, half], BF16, tag="y2", bufs=2,
                                           name=f"y2_{hx}")
                            nc.scalar.activation(
                                y[:], ps[:], mybir.ActivationFunctionType.Copy,
                            )
                            nc.sync.dma_start(
                                out=ytp[dsl, t0 + hx * half:t0 + (hx + 1) * half],
                                in_=y[:],
                            )
                        continue
                    ps = psum.tile([128, TT], F32, tag="ps", name=f"ps2_{tt}_{db}")
                    for k in range(NK2):
                        nc.tensor.matmul(
                            ps[:],
                            dn_sb[k][:, db * 128:(db + 1) * 128],
                            h_sb[tt][k][:, :],
                            start=(k == 0),
                            stop=(k == NK2 - 1),
                        )
                    y = ypool.tile([128, TT], BF16, tag="y", name=f"y_{tt}_{db}")
                    nc.scalar.activation(
                        y[:], ps[:], mybir.ActivationFunctionType.Copy,
                    )
                    nc.sync.dma_start(
                        out=ytp[db * 128:(db + 1) * 128, t0:t0 + TT], in_=y[:],
                    )

            gemm1_ksweep(0, list(range(8)), kchunk=2)
            gemm1_plain(0, list(range(8, M1)))
            gemm1_plain(1, list(range(M1)))
            gemm2(0, list(range(M2)))
            gemm2(1, list(range(M2)), split_last=True)

    _split_multi_waits(nc)
    nc.finalize()
    return nc


def _get_nc(warm: int) -> bass.Bass:
    if warm not in _CACHE:
        _CACHE[warm] = build_nc(warm)
    return _CACHE[warm]


def kernel(x, gate_w, up_w, down_w):
    global LAST_RESULTS
    from concourse.bass_utils import run_bass_kernel_spmd

    warm = int(os.environ.get("MOE_WARM", "0"))
    nc = _get_nc(warm)

    xf = np.asarray(x, dtype=np.float32).reshape(T, D)
    xT = np.ascontiguousarray(xf.T).astype(NP_BF16)            # [D, T]
    upT = np.ascontiguousarray(np.asarray(up_w, dtype=np.float32).T).astype(NP_BF16)
    dnT = np.ascontiguousarray(np.asarray(down_w, dtype=np.float32).T).astype(NP_BF16)

    in_maps = []
    for c in range(NC_CORES):
        in_maps.append({
            "xt": np.ascontiguousarray(xT[:, c * TC:(c + 1) * TC]),
            "upw": upT,
            "dwn": dnT,
        })

    res = run_bass_kernel_spmd(nc, in_maps, list(range(NC_CORES)))
    LAST_RESULTS = res

    out = np.empty((T, D), dtype=np.float32)
    for c in range(NC_CORES):
        out[c * TC:(c + 1) * TC, :] = res.results[c]["ytp"].T.astype(np.float32)
    return out.reshape(B, S, D)


# revision 6
# speedup vs baseline: 1.4361x; 1.1828x over previous
"""MoE (single shared expert) kernel for 8 trn2 NeuronCores.

Math: the reference's top-2 gating over 64 "experts" feeds a single shared
FFN, and the renormalized top-2 weights sum to s/(s+1e-9) with s >= 1/64,
i.e. 1 up to <= 6.4e-8 relative -- below f32 rounding noise.  The whole
module therefore reduces to:  out = silu(x @ up_w.T) @ down_w.T.

Dtype strategy (all measured on this silicon, 512-col matmuls):
  moving operand rate:  fp8-DR 216ns | f32r 230ns | bf16 260ns / instr
  fp8 needs 6 residual-corrected DoubleRow passes to meet 2e-2 rel err
  (one raw fp8 tensor alone costs ~2.7e-2), i.e. 162us PE -- dead.
So: moving operands (x, h) stay f32r, stationary weights are bf16 (halves
the weight DMA; LDWEIGHTS fully overlaps matmuls either way), output is
written bf16 and upcast on host.  PE floor: 512 instrs x 230ns = 117.8us.

Sharding (8 cores): token-parallel, 1024 tokens/core, weights replicated.
DMA order (up[k] bf16 | x_tt0[k]) x8 -> x_tt1 -> dn, with GEMM1's first
8 m-tiles run as k=1 sweeps across 8 PSUM banks so the PE starts after
only up[0]+x[0] = 0.75MB of DMA.  GEMM2's last m-tile is split into
column halves to shorten the tail.
"""

import os
import sys

import numpy as np
import ml_dtypes

for _p in ("/opt/trn_rl_repo",):
    if os.path.isdir(_p) and _p not in sys.path:
        sys.path.insert(0, _p)

import concourse.bass as bass
import concourse.mybir as mybir
import concourse.tile as tile

F32 = mybir.dt.float32
F32R = mybir.dt.float32r
BF16 = mybir.dt.bfloat16
NP_BF16 = ml_dtypes.bfloat16


def _ensure_axon_hooks_shim():
    """bass_utils' trace path imports antenv.axon_hooks, which this image
    lacks; give it a no-op hook module so BASS_TRACE=1 degrades gracefully."""
    import types
    if "antenv.axon_hooks" in sys.modules:
        return
    try:
        import antenv
    except ImportError:
        return
    if hasattr(antenv, "axon_hooks"):
        return
    ah = types.ModuleType("antenv.axon_hooks")
    ah._hook = None
    ah.set_axon_ntff_profile_hook = lambda h: setattr(ah, "_hook", h)
    ah.get_axon_ntff_profile_hook = lambda: ah._hook
    sys.modules["antenv.axon_hooks"] = ah
    antenv.axon_hooks = ah


_ensure_axon_hooks_shim()


def _split_multi_waits(nc):
    """This container's walrus encodes at most ONE sync wait per engine
    instruction ("Too many sync wait commands").  Tile routinely emits
    instructions waiting on several semaphores; hoist the extra waits onto
    single-wait NoOps inserted just before, on the same engine."""
    n = 0
    for f in nc.m.functions:
        for blk in f.blocks:
            insts = blk.instructions
            out = []
            for inst in insts:
                si = inst.sync_info
                waits = list(si.on_wait) if si and si.on_wait else []
                if len(waits) > 1:
                    for w in waits[:-1]:
                        n += 1
                        nop = mybir.InstNoOp(name=f"I-wsplit-{n}", ins=[], outs=[])
                        nop.engine = inst.engine
                        nop.sync_info = mybir.SyncInfo(on_wait=[w], on_update=[])
                        nc.register_instruction(nop)
                        out.append(nop)
                    si.on_wait = [waits[-1]]
                out.append(inst)
            if n:
                insts[:] = out
    return n


# Problem shape (hardcoded per contract)
B, S, D, ED = 4, 2048, 1024, 2048
T = B * S                     # 8192 tokens
NC_CORES = 8
TC = T // NC_CORES            # tokens per core = 1024
TT = 512                      # token tile (matmul moving free dim)
NTT = TC // TT                # 2 token tiles
NK1 = D // 128                # 8 k-tiles for GEMM1
NK2 = ED // 128               # 16 k-tiles for GEMM2
M1 = ED // 128                # 16 output e-tiles of GEMM1
M2 = D // 128                 # 8 output d-tiles of GEMM2

_CACHE = {}
LAST_RESULTS = None           # BassKernelResults of the most recent run


def build_nc(warm: int = 0, dt: str = "f16") -> bass.Bass:
    """One-core SPMD program: ytp[D, TC] = (silu(x @ upT) @ dwnT).T."""
    MDT = {"bf16": BF16, "f16": mybir.dt.float16, "f32": F32R}[dt]
    nc = bass.Bass()
    xt = nc.dram_tensor("xt", [D, TC], MDT, kind="ExternalInput")
    upw = nc.dram_tensor("upw", [D, ED], MDT, kind="ExternalInput")
    dwn = nc.dram_tensor("dwn", [ED, D], MDT, kind="ExternalInput")
    ytp = nc.dram_tensor("ytp", [D, TC], BF16, kind="ExternalOutput")

    with tile.TileContext(nc) as tc:
        with (
            tc.tile_pool(name="wpool", bufs=1) as wpool,
            tc.tile_pool(name="hpool", bufs=6) as hpool,
            tc.tile_pool(name="ypool", bufs=4) as ypool,
            tc.tile_pool(name="psum", bufs=8, space="PSUM") as psum,
        ):
            up_sb = [wpool.tile([128, ED], MDT, tag=f"up{k}", name=f"up{k}")
                     for k in range(NK1)]
            # x tiles split per (k, token-tile) so tt1's bytes can stream late
            x_sb = [[wpool.tile([128, TT], MDT, tag=f"x{k}_{tt}", name=f"x{k}_{tt}")
                     for tt in range(NTT)] for k in range(NK1)]
            dn_sb = [wpool.tile([128, D], MDT, tag=f"dn{k}", name=f"dn{k}")
                     for k in range(NK2)]
            h_sb = [[wpool.tile([128, TT], MDT, tag=f"h{tt}_{e}", name=f"h{tt}_{e}")
                     for e in range(M1)] for tt in range(NTT)]

            # Optional PE pre-ramp: dependency-free matmuls issued at t=0 so
            # the HAM clock reaches 2.4GHz while the first DMAs stream.
            if warm:
                wz = wpool.tile([128, 128], MDT, tag="warmw", name="warmw")
                xz = wpool.tile([128, TT], MDT, tag="warmx", name="warmx")
                nc.vector.memset(wz[:], 0.0)
                nc.vector.memset(xz[:], 0.0)
                wps = psum.tile([128, TT], F32, tag="ps", name="warm_ps")
                for _ in range(warm):
                    nc.tensor.matmul(wps[:], wz[:], xz[:], start=True, stop=True)
                wsink = ypool.tile([128, TT], F32, tag="wsink", name="warm_sink")
                nc.vector.tensor_copy(wsink[:], wps[:])

            # DMA emission in consumption order.
            for k in range(NK1):
                nc.sync.dma_start(out=up_sb[k][:], in_=upw[k * 128:(k + 1) * 128, :])
                nc.sync.dma_start(out=x_sb[k][0][:],
                                  in_=xt[k * 128:(k + 1) * 128, 0:TT])
            for k in range(NK1):
                nc.sync.dma_start(out=x_sb[k][1][:],
                                  in_=xt[k * 128:(k + 1) * 128, TT:TC])
            for k in range(NK2):
                nc.sync.dma_start(out=dn_sb[k][:], in_=dwn[k * 128:(k + 1) * 128, :])

            def evac1(tt, ei, ps):
                sg = hpool.tile([128, TT], F32, tag="sg", bufs=4,
                                name=f"sg_{tt}_{ei}")
                nc.scalar.activation(
                    sg[:], ps[:], mybir.ActivationFunctionType.Sigmoid,
                )
                nc.vector.tensor_mul(h_sb[tt][ei][:], ps[:], sg[:])

            def gemm1_plain(tt, eis):
                for ei in eis:
                    ps = psum.tile([128, TT], F32, tag="ps", name=f"ps1_{tt}_{ei}")
                    for k in range(NK1):
                        nc.tensor.matmul(
                            ps[:],
                            up_sb[k][:, ei * 128:(ei + 1) * 128],
                            x_sb[k][tt][:],
                            start=(k == 0),
                            stop=(k == NK1 - 1),
                        )
                    evac1(tt, ei, ps)

            def gemm1_ksweep(tt, eis, kchunk=1):
                """k-chunked sweeps across len(eis) concurrent PSUM banks so
                the first matmul only needs up[0:kchunk] + x[0:kchunk]."""
                pss = [psum.tile([128, TT], F32, tag="ps", name=f"ps1_{tt}_{ei}")
                       for ei in eis]
                for k0 in range(0, NK1, kchunk):
                    for j, ei in enumerate(eis):
                        for k in range(k0, k0 + kchunk):
                            nc.tensor.matmul(
                                pss[j][:],
                                up_sb[k][:, ei * 128:(ei + 1) * 128],
                                x_sb[k][tt][:],
                                start=(k == 0),
                                stop=(k == NK1 - 1),
                            )
                for j, ei in enumerate(eis):
                    evac1(tt, ei, pss[j])

            def gemm2(tt, dbs, split_last=False):
                t0 = tt * TT
                for db in dbs:
                    if split_last and db == dbs[-1]:
                        # shorten the kernel tail: column halves so the first
                        # half's evac+DMA overlap the second half's matmuls
                        dsl = slice(db * 128, (db + 1) * 128)
                        half = TT // 2
                        for hx in range(2):
                            ps = psum.tile([128, half], F32, tag="ps",
                                           name=f"ps2_last_{hx}")
                            for k in range(NK2):
                                nc.tensor.matmul(
                                    ps[:],
                                    dn_sb[k][:, dsl],
                                    h_sb[tt][k][:, hx * half:(hx + 1) * half],
                                    start=(k == 0),
                                    stop=(k == NK2 - 1),
                                )
                            y = ypool.tile([128, half], BF16, tag="y2", bufs=2,
                                           name=f"y2_{hx}")
                            nc.scalar.activation(
                                y[:], ps[:], mybir.ActivationFunctionType.Copy,
                            )
                            nc.sync.dma_start(
                                out=ytp[dsl, t0 + hx * half:t0 + (hx + 1) * half],
                                in_=y[:],
                            )
                        continue
                    ps = psum.tile([128, TT], F32, tag="ps", name=f"ps2_{tt}_{db}")
                    for k in range(NK2):
                        nc.tensor.matmul(
                            ps[:],
                            dn_sb[k][:, db * 128:(db + 1) * 128],
                            h_sb[tt][k][:],
                            start=(k == 0),
                            stop=(k == NK2 - 1),
                        )
                    y = ypool.tile([128, TT], BF16, tag="y", name=f"y_{tt}_{db}")
                    nc.scalar.activation(
                        y[:], ps[:], mybir.ActivationFunctionType.Copy,
                    )
                    nc.sync.dma_start(
                        out=ytp[db * 128:(db + 1) * 128, t0:t0 + TT], in_=y[:],
                    )

            gemm1_ksweep(0, list(range(8)), kchunk=1)
            gemm1_plain(0, list(range(8, M1)))
            gemm1_plain(1, list(range(M1)))
            gemm2(0, list(range(M2)))
            gemm2(1, list(range(M2)), split_last=True)

    _split_multi_waits(nc)
    nc.finalize()
    return nc


def _get_nc(warm: int, dt: str) -> bass.Bass:
    key = (warm, dt)
    if key not in _CACHE:
        _CACHE[key] = build_nc(warm, dt)
    return _CACHE[key]


def kernel(x, gate_w, up_w, down_w):
    global LAST_RESULTS
    from concourse.bass_utils import run_bass_kernel_spmd

    warm = int(os.environ.get("MOE_WARM", "0"))
    dt = os.environ.get("MOE_DT", "f16")
    nc = _get_nc(warm, dt)

    np_dt = {"bf16": NP_BF16, "f16": np.float16, "f32": np.float32}[dt]
    xf = np.asarray(x, dtype=np.float32).reshape(T, D)
    xT = np.ascontiguousarray(xf.T).astype(np_dt)              # [D, T]
    upT = np.ascontiguousarray(np.asarray(up_w, dtype=np.float32).T).astype(np_dt)
    dnT = np.ascontiguousarray(np.asarray(down_w, dtype=np.float32).T).astype(np_dt)

    in_maps = []
    for c in range(NC_CORES):
        in_maps.append({
            "xt": np.ascontiguousarray(xT[:, c * TC:(c + 1) * TC]),
            "upw": upT,
            "dwn": dnT,
        })

    res = run_bass_kernel_spmd(nc, in_maps, list(range(NC_CORES)))
    LAST_RESULTS = res

    out = np.empty((T, D), dtype=np.float32)
    for c in range(NC_CORES):
        out[c * TC:(c + 1) * TC, :] = res.results[c]["ytp"].T.astype(np.float32)
    return out.reshape(B, S, D)
